# revision 33
# baseline (speedup 1.0000x reference)
"""Trainium2 Bass kernel for nn_BlockBlastValueNet1PmultikernelFlattenned.

Strategy (v2)
-------------
The network is 8 tiny conv branches over an 8x8 board followed by small MLPs.
Every conv branch (pad const 1.0 + valid conv + bias) is an affine map of the
64 board values, so the whole net folds into:

    y  = x @ W1                          # [B, NF]  (NF = 2944, no bias in psum)
    h  = W12 x + W2s.T ev(y) + b2f       # ev = per-tile relu-ish evacuation
    g1 = Lrelu( h @ W3 + b3 )
    g2 = Lrelu( g1 @ W4 + b4 )           # augmented with a ones column
    out = g2 @ W5                        # fc3 (bias folded via augmentation)

Key structure vs v1: W2 is BLOCK-DIAGONAL (8 branches x 16 h-outputs).  The
step-2 contraction runs as 4 independent chains in the four 32-column strips
of the PE array (tile_position=(0|64, 32g)); the 4 chains execute
CONCURRENTLY, so step-2 costs ~max-chain (9 streams) instead of 23+ serial
K-tile streams.  Branch pairs are laid out in 64-row-aligned K-groups so a
K-tile shared by two groups splits at the array row-half boundary (top half
one strip, bottom half another - still concurrent).

The Lrelu between the two big matmuls is decomposed as
    Lrelu(v + c1) = 0.01 v + 0.99 ev + const
where the evacuated tensor ev is per-tile either
    r = relu(y + c1)        (Scalar/ACT engine: 1-op activation w/ bias)
    z = max(y, -c1)         (Vector/DVE engine: 1-op tensor_scalar_max)
(z = r - c1, so the difference folds into the h bias b2f with per-tile
coefficient kappa: 0.01 for r-tiles, 1.00 for z-tiles).  The 0.01*v path
collapses into a 64->128 fold matmul (W12) opening each pair's h-accumulator
(single full-width start=True clears the psum bank; the 4 strip chains then
accumulate start=False).

PSUM->SBUF evacuation is the hard wall: only DVE and ACT can read PSUM
(GPSIMD has no PSUM port on trn2, DMA none either), both at 1 elem/cycle/
partition for fp32 sources.  23 tiles x [128,1024] per pair ~= 14us across
both engines; everything else is scheduled to hide under that.

Data-parallel over 8 NeuronCores (batch 65536 -> 8192/core), feature-major
layout, samples processed in pairs of 512-chunks (psum-bank sized).  All
matmuls fp16 (full PE rate, keeps the PE activity monitor warm).
"""

import numpy as np

# ---------------------------------------------------------------- constants
SPECS = [(1, 1, 1, 0, 0), (2, 2, 6, 1, 1), (3, 3, 8, 1, 1), (4, 4, 8, 2, 2),
         (5, 5, 16, 2, 2), (8, 8, 32, 0, 0), (1, 8, 4, 0, 0), (8, 1, 4, 0, 0)]
BOARD = 8
B_TOTAL = 65536
N_CORES = 8
BC = B_TOTAL // N_CORES          # 8192 samples per core
PAIR_N = 1024                    # samples per pair-iteration (2 psum banks)
CHUNK = 512                      # matmul moving width (1 psum bank fp32)
N_PAIRS = BC // PAIR_N           # 8
LRELU_NEG = 0.01

# column-group pairing of branches (2 branches x 16 h-outputs = 32 cols each)
GROUPS = [[4, 5], [3, 6], [2, 7], [1, 0]]
_BR_N = []
for kh, kw, fs, ph, pw in SPECS:
    _BR_N.append((BOARD + 2 * ph - kh + 1) * (BOARD + 2 * pw - kw + 1) * fs)

# 64-aligned group K-offsets so straddle tiles split at the row-half boundary
G_OFF, G_END = [], []
_o = 0
for g in GROUPS:
    G_OFF.append(_o)
    _o += sum(_BR_N[b] for b in g)
    G_END.append(_o)
    _o = -(-_o // 64) * 64       # round up to 64
KT = -(-_o // 128)               # 23 K-tiles
NF = KT * 128                    # 2944

# per-group stream lists: (tile, kind) with kind in {'full','top','bot'}
G_STREAMS = []
for gi in range(4):
    a = G_OFF[gi]
    b = -(-G_END[gi] // 64) * 64
    st = []
    for t in range(a // 128, -(-b // 128)):
        lo, hi = max(a, 128 * t), min(b, 128 * t + 128)
        if hi - lo == 128:
            st.append((t, 'full'))
        elif lo == 128 * t:
            st.append((t, 'top'))
        else:
            st.append((t, 'bot'))
    G_STREAMS.append(st)
TILE_STREAMS = {}                # tile -> [(group, kind)]
for gi in range(4):
    for t, kind in G_STREAMS[gi]:
        TILE_STREAMS.setdefault(t, []).append((gi, kind))
# per-strip fold openers (the 0.01*x path): first stream of each chain,
# start=True clears exactly the partitions the chain accumulates into.
# Row-half alternates so openers of adjacent strips are fully disjoint.
FOLD_ROW = [64, 0, 64, 0]
N_STREAMS = sum(len(s) for s in G_STREAMS) + 4   # 25 + folds

# step-1 slot -> (psA tile, psB tile).  psA tiles evacuate on DVE (z-form),
# psB tiles on ACT (r-form).  Order interleaves groups so the 4 step-2
# chains get tiles round-robin.
PSA = [0, 14, 1, 15, 2, 16, 3, 17, 4, 18, 6, 7]
PSB = [9, 19, 10, 20, 11, 21, 12, 22, 13, 5, 8, None]
N_S1 = len(PSA)                  # 12
assert sorted([t for t in PSA + PSB if t is not None]) == list(range(KT))
DVE_TILES = set(PSA)
LAG = 2                          # slots between evac emission and chain use

# consume streams in tile-production order (chain accumulation commutes)
_PROD = {}
for _s in range(N_S1):
    _PROD[PSA[_s]] = _s
    if PSB[_s] is not None:
        _PROD[PSB[_s]] = _s
for _gi in range(4):
    G_STREAMS[_gi].sort(key=lambda tk: _PROD[tk[0]])
    G_STREAMS[_gi].insert(0, (None, 'fold'))   # opener; no y-tile dep


# ---------------------------------------------------------------- host fold
def _fold_params(p):
    """Fold conv branches + MLPs into the dense pipeline weights (float64)."""
    n_of = _BR_N
    W1_of, c1_of = {}, {}
    for i, (kh, kw, fs, ph, pw) in enumerate(SPECS):
        Ho = BOARD + 2 * ph - kh + 1
        Wo = BOARD + 2 * pw - kw + 1
        cw = np.asarray(p[f"b{i}_cw"], np.float64)
        cb = np.asarray(p[f"b{i}_cb"], np.float64)
        W1 = np.zeros((64, n_of[i]))
        c1 = np.zeros((n_of[i],))
        for f in range(fs):
            for oh in range(Ho):
                for ow in range(Wo):
                    oi = (f * Ho + oh) * Wo + ow
                    c1[oi] += cb[f]
                    for u in range(kh):
                        for v in range(kw):
                            r, c = oh + u - ph, ow + v - pw
                            w = cw[f, 0, u, v]
                            if 0 <= r < 8 and 0 <= c < 8:
                                W1[r * 8 + c, oi] += w
                            else:
                                c1[oi] += w        # pad value is 1.0
        W1_of[i] = W1
        c1_of[i] = c1

    # K-layout: 64-aligned groups; h block order = group order
    K_start = {}
    for gi, g in enumerate(GROUPS):
        off = G_OFF[gi]
        for b in g:
            K_start[b] = off
            off += n_of[b]
    border = [b for g in GROUPS for b in g]
    hpos = {b: j * 16 for j, b in enumerate(border)}

    W1p = np.zeros((64, NF))
    c1p = np.zeros((NF,))
    W2p = np.zeros((NF, 128))
    b2p = np.zeros((128,))
    for b in range(8):
        s, n, hp = K_start[b], n_of[b], hpos[b]
        W1p[:, s:s + n] = W1_of[b]
        c1p[s:s + n] = c1_of[b]
        W2p[s:s + n, hp:hp + 16] = np.asarray(p[f"b{b}_w1"], np.float64).T
        b2p[hp:hp + 16] = np.asarray(p[f"b{b}_b1"], np.float64)

    Wb = np.zeros((128, 64))
    bb = np.zeros((64,))
    for b in range(8):
        hp = hpos[b]
        Wb[hp:hp + 16, 8 * b:8 * b + 8] = np.asarray(p[f"b{b}_w2"], np.float64).T
        bb[8 * b:8 * b + 8] = np.asarray(p[f"b{b}_b2"], np.float64)
    fc_w1 = np.asarray(p["fc_w1"], np.float64)
    fc_b1 = np.asarray(p["fc_b1"], np.float64)
    W3 = Wb @ fc_w1.T
    b3 = bb @ fc_w1.T + fc_b1
    fc_w2 = np.asarray(p["fc_w2"], np.float64)
    fc_b2 = np.asarray(p["fc_b2"], np.float64)
    fc_w3 = np.asarray(p["fc_w3"], np.float64)
    fc_b3 = np.asarray(p["fc_b3"], np.float64)
    W4 = np.zeros((64, 17)); W4[:, :16] = fc_w2.T
    b4 = np.zeros((17,)); b4[:16] = fc_b2; b4[16] = 1.0
    W5 = np.zeros((17, 1)); W5[:16, 0] = fc_w3[0]; W5[16, 0] = fc_b3[0]

    # relu decomposition folds: Lrelu(y+c1) = 0.01(y+c1) + 0.99 relu(y+c1)
    # DVE tiles evacuate z = max(y,-c1) = relu(y+c1) - c1  -> kappa = 1.00
    # ACT tiles evacuate r = relu(y+c1)                    -> kappa = 0.01
    W2s = (1.0 - LRELU_NEG) * W2p
    W12 = LRELU_NEG * (W1p @ W2p)
    b2f = b2p.copy()
    for t in range(KT):
        kap = 1.0 if t in DVE_TILES else LRELU_NEG
        b2f += kap * (c1p[128 * t:128 * (t + 1)] @ W2p[128 * t:128 * (t + 1)])

    f32 = np.float32
    f16 = np.float16
    dev = {}
    # step-1 weights: slot s holds M-tiles PSA[s] (rows 0:64) | PSB[s] (64:128)
    w1 = np.zeros((128, N_S1, 128), f16)
    for s in range(N_S1):
        w1[0:64, s, :] = W1p[:, 128 * PSA[s]:128 * (PSA[s] + 1)]
        if PSB[s] is not None:
            w1[64:128, s, :] = W1p[:, 128 * PSB[s]:128 * (PSB[s] + 1)]
    dev["w1"] = w1
    c1t = np.zeros((128, KT), f32)      # ACT bias (+c1)
    nc1t = np.zeros((128, KT), f32)     # DVE max operand (-c1)
    for t in range(KT):
        c1t[:, t] = c1p[128 * t:128 * (t + 1)]
        nc1t[:, t] = -c1p[128 * t:128 * (t + 1)]
    dev["c1t"] = c1t
    dev["nc1t"] = nc1t
    w2 = np.zeros((128, KT, 128), f16)
    for t in range(KT):
        w2[:, t, :] = W2s[128 * t:128 * (t + 1), :]
    dev["w2"] = w2
    dev["w12"] = np.vstack([W12, W12]).astype(f16)  # both row-halves
    dev["b2f"] = b2f.reshape(128, 1).astype(f32)
    dev["w3"] = W3.astype(f16)
    dev["b3"] = b3.reshape(64, 1).astype(f32)
    dev["w4"] = W4.astype(f16)
    dev["b4"] = b4.reshape(17, 1).astype(f32)
    dev["w5"] = W5.astype(f16)
    return dev


# ---------------------------------------------------------------- device IR
def _build_nc(n_pairs=N_PAIRS):
    import concourse.mybir as mybir
    import concourse.tile as tile
    from concourse import bacc
    from contextlib import ExitStack

    dt = mybir.dt
    AF = mybir.ActivationFunctionType
    f32 = dt.float32
    f16 = dt.float16
    bc = n_pairs * PAIR_N

    nc = bacc.Bacc("TRN2", target_bir_lowering=False, debug=False,
                   num_devices=N_CORES)

    xx_d = nc.dram_tensor("xx", [128, bc], f16, kind="ExternalInput")
    w1_d = nc.dram_tensor("w1", [128, N_S1, 128], f16, kind="ExternalInput")
    c1t_d = nc.dram_tensor("c1t", [128, KT], f32, kind="ExternalInput")
    nc1t_d = nc.dram_tensor("nc1t", [128, KT], f32, kind="ExternalInput")
    w2_d = nc.dram_tensor("w2", [128, KT, 128], f16, kind="ExternalInput")
    w12_d = nc.dram_tensor("w12", [128, 128], f16, kind="ExternalInput")
    b2f_d = nc.dram_tensor("b2f", [128, 1], f32, kind="ExternalInput")
    w3_d = nc.dram_tensor("w3", [128, 64], f16, kind="ExternalInput")
    b3_d = nc.dram_tensor("b3", [64, 1], f32, kind="ExternalInput")
    w4_d = nc.dram_tensor("w4", [64, 17], f16, kind="ExternalInput")
    b4_d = nc.dram_tensor("b4", [17, 1], f32, kind="ExternalInput")
    w5_d = nc.dram_tensor("w5", [17, 1], f16, kind="ExternalInput")
    o_d = nc.dram_tensor("o", [1, bc], f32, kind="ExternalOutput")

    with tile.TileContext(nc) as tc, ExitStack() as ctx:
        wpool = ctx.enter_context(tc.tile_pool(name="wpool", bufs=1))
        xpool = ctx.enter_context(tc.tile_pool(name="xpool", bufs=3))
        ypool = ctx.enter_context(tc.tile_pool(name="ypool", bufs=KT + 3))
        spool = ctx.enter_context(tc.tile_pool(name="spool", bufs=2))
        ps1p = ctx.enter_context(tc.tile_pool(name="ps1p", bufs=3, space="PSUM"))
        ps2p = ctx.enter_context(tc.tile_pool(name="ps2p", bufs=1, space="PSUM"))

        # pair-0 input first so compute can start while the rest streams in
        xx_first = xpool.tile([128, PAIR_N], f16, tag="xx", name="xx_first")
        nc.sync.dma_start(xx_first[:], xx_d[:, 0:PAIR_N])
        w1_t = wpool.tile([128, N_S1, 128], f16)
        nc.sync.dma_start(w1_t[:], w1_d[:])
        w2_t = wpool.tile([128, KT, 128], f16)
        nc.scalar.dma_start(w2_t[:], w2_d[:])
        c1t_t = wpool.tile([128, KT], f32)
        nc.scalar.dma_start(c1t_t[:], c1t_d[:])
        nc1t_t = wpool.tile([128, KT], f32)
        nc.sync.dma_start(nc1t_t[:], nc1t_d[:])
        w12_t = wpool.tile([128, 128], f16)
        nc.gpsimd.dma_start(w12_t[:], w12_d[:])
        b2f_t = wpool.tile([128, 1], f32)
        nc.gpsimd.dma_start(b2f_t[:], b2f_d[:])
        w3_t = wpool.tile([128, 64], f16)
        nc.gpsimd.dma_start(w3_t[:], w3_d[:])
        b3_t = wpool.tile([64, 1], f32)
        nc.gpsimd.dma_start(b3_t[:], b3_d[:])
        w4_t = wpool.tile([64, 17], f16)
        nc.gpsimd.dma_start(w4_t[:], w4_d[:])
        b4_t = wpool.tile([17, 1], f32)
        nc.gpsimd.dma_start(b4_t[:], b4_d[:])
        w5_t = wpool.tile([17, 1], f16)
        nc.gpsimd.dma_start(w5_t[:], w5_d[:])

        # PE warm-up: dependency-free dummy matmuls run while the input
        # DMAs land, so the HAM un-throttles (K=8/8 @ 2.4GHz) before the
        # first real slot instead of ~4us into it.  Results are discarded
        # (the scratch psum is never read; real slots start=True overwrite).
        scratch = wpool.tile([128, CHUNK], f16, name="warmup_src")
        nc.vector.memset(scratch[:], 0)
        wu_ps = ps1p.tile([128, PAIR_N], f32, tag="ps1", name="warmup_ps")
        for _wu in range(32):
            nc.tensor.matmul(wu_ps[:, 0:128], scratch[0:64, 0:128],
                             scratch[0:64, 0:128], start=True, stop=True,
                             tile_position=(0, 0))

        class ChainState:
            """Per-pair step-2 chain bookkeeping (survives into next pair)."""
            def __init__(self, ps2, ytiles, xx):
                self.ps2 = ps2
                self.ytiles = ytiles
                self.xx = xx
                self.q = [list(s) for s in G_STREAMS]  # pending per group
                self.avail = set()                     # tiles evac'd (lagged)
                self.left = N_STREAMS

            def ready(self, gi):
                if not self.q[gi]:
                    return False
                t = self.q[gi][0][0]
                return t is None or t in self.avail

            def empty(self):
                return not any(self.q)

        def emit_wave(st):
            """Emit one concurrent wave: <=1 stream per group, interleaved
            chunks so the 4 col-strips run concurrently on the PE."""
            popped = []
            for gi in range(4):
                if st.ready(gi):
                    popped.append((gi,) + st.q[gi].pop(0))
            if not popped:
                return False
            st.left -= len(popped)
            last = st.left == 0
            for h in range(2):
                sl = slice(h * CHUNK, (h + 1) * CHUNK)
                for i, (gi, t, kind) in enumerate(popped):
                    cs = slice(32 * gi, 32 * gi + 32)
                    stop = last and i == len(popped) - 1
                    if kind == 'fold':
                        r0 = FOLD_ROW[gi]
                        nc.tensor.matmul(
                            st.ps2[cs, sl], w12_t[r0:r0 + 64, cs],
                            st.xx[r0:r0 + 64, sl], start=True, stop=stop,
                            tile_position=(r0, 32 * gi),
                            skip_group_check=True)
                    elif kind == 'full':
                        nc.tensor.matmul(
                            st.ps2[cs, sl], w2_t[:, t, cs],
                            st.ytiles[t][:, sl], start=False, stop=stop,
                            tile_position=(0, 32 * gi), skip_group_check=True)
                    elif kind == 'top':
                        nc.tensor.matmul(
                            st.ps2[cs, sl], w2_t[0:64, t, cs],
                            st.ytiles[t][0:64, sl], start=False, stop=stop,
                            tile_position=(0, 32 * gi), skip_group_check=True)
                    else:
                        nc.tensor.matmul(
                            st.ps2[cs, sl], w2_t[64:128, t, cs],
                            st.ytiles[t][64:128, sl], start=False, stop=stop,
                            tile_position=(64, 32 * gi), skip_group_check=True)
            return True

        def make_tail_stages(p, ps2):
            """Per-pair serial tail (h -> g1 -> g2 -> out), interleaved into
            the NEXT pair's slot stream."""
            st = {}

            def s0b():
                st["h"] = spool.tile([128, PAIR_N], f16, tag="h", name=f"h_{p}")
                nc.scalar.activation(st["h"][:], ps2[:], AF.Lrelu,
                                     bias=b2f_t[:, 0:1], alpha=LRELU_NEG)

            def s1():
                st["g1ps"] = ps1p.tile([64, PAIR_N], f32, tag="ps1",
                                       name=f"g1ps_{p}")
                for h in range(2):
                    sl = slice(h * CHUNK, (h + 1) * CHUNK)
                    nc.tensor.matmul(st["g1ps"][0:32, sl], w3_t[:, 0:32],
                                     st["h"][:, sl], start=True, stop=True,
                                     tile_position=(0, 0),
                                     skip_group_check=True)
                    nc.tensor.matmul(st["g1ps"][32:64, sl], w3_t[:, 32:64],
                                     st["h"][:, sl], start=True, stop=True,
                                     tile_position=(0, 32),
                                     skip_group_check=True)

            def s2():
                st["g1"] = spool.tile([64, PAIR_N], f16, tag="g1",
                                      name=f"g1_{p}")
                nc.scalar.activation(st["g1"][:], st["g1ps"][:], AF.Lrelu,
                                     bias=b3_t[:, 0:1], alpha=LRELU_NEG)

            def s3():
                # g2ps and the final output row share one psum tile
                st["tps"] = ps1p.tile([33, PAIR_N], f32, tag="ps1",
                                      name=f"tps_{p}")
                for h in range(2):
                    sl = slice(h * CHUNK, (h + 1) * CHUNK)
                    nc.tensor.matmul(st["tps"][0:17, sl], w4_t[:],
                                     st["g1"][:, sl], start=True, stop=True)

            def s4():
                st["g2"] = spool.tile([17, PAIR_N], f16, tag="g2",
                                      name=f"g2_{p}")
                nc.scalar.activation(st["g2"][:], st["tps"][0:17, :], AF.Lrelu,
                                     bias=b4_t[:, 0:1], alpha=LRELU_NEG)

            def s5():
                for h in range(2):
                    sl = slice(h * CHUNK, (h + 1) * CHUNK)
                    nc.tensor.matmul(st["tps"][32:33, sl], w5_t[:],
                                     st["g2"][:, sl], start=True, stop=True,
                                     tile_position=(0, 32),
                                     skip_group_check=True)

            def s6():
                o_t = spool.tile([1, PAIR_N], f32, tag="o", name=f"o_{p}")
                nc.vector.tensor_copy(o_t[:, 0:CHUNK],
                                      st["tps"][32:33, 0:CHUNK])
                nc.scalar.activation(o_t[:, CHUNK:PAIR_N],
                                     st["tps"][32:33, CHUNK:PAIR_N], AF.Copy)
                nc.sync.dma_start(o_d[:, p * PAIR_N:(p + 1) * PAIR_N], o_t[:])

            return [s0b, None, s1, s2, None, s3, s4, None, s5, s6]

        tail_stages = []
        prev_chain = None

        for p in range(n_pairs):
            if p == 0:
                xx_t = xx_first
            else:
                xx_t = xpool.tile([128, PAIR_N], f16, tag="xx", name=f"xx_{p}")
                nc.sync.dma_start(xx_t[:],
                                  xx_d[:, p * PAIR_N:(p + 1) * PAIR_N])

            ytiles = [None] * KT
            chain = ChainState(None, ytiles, xx_t)

            pend = []
            for s in range(N_S1):
                tA, tB = PSA[s], PSB[s]
                psA = ps1p.tile([128, PAIR_N], f32, tag="ps1",
                                name=f"psA_{p}_{s}")
                if tB is not None:
                    psB = ps1p.tile([128, PAIR_N], f32, tag="ps1",
                                    name=f"psB_{p}_{s}")
                for h in range(2):
                    sl = slice(h * CHUNK, (h + 1) * CHUNK)
                    nc.tensor.matmul(psA[:, sl], w1_t[0:64, s, :],
                                     xx_t[0:64, sl], start=True, stop=True,
                                     tile_position=(0, 0))
                    if tB is not None:
                        nc.tensor.matmul(psB[:, sl], w1_t[64:128, s, :],
                                         xx_t[64:128, sl], start=True,
                                         stop=True, tile_position=(64, 0))

                # evacuations first so they are never queued behind a
                # ~1.1us tail-stage op on the same engine
                yA = ypool.tile([128, PAIR_N], f16, tag="y", name=f"y_{p}_{tA}")
                nc.vector.tensor_scalar_max(yA[:], psA[:],
                                            nc1t_t[:, tA:tA + 1])
                ytiles[tA] = yA
                done = [tA]
                if tB is not None:
                    yB = ypool.tile([128, PAIR_N], f16, tag="y",
                                    name=f"y_{p}_{tB}")
                    nc.scalar.activation(yB[:], psB[:], AF.Relu,
                                         bias=c1t_t[:, tB:tB + 1])
                    ytiles[tB] = yB
                    done.append(tB)

                if s == 0 and prev_chain is not None:
                    # finish previous pair's chains (queued behind this
                    # slot's step-1 MMs, so their evac waits don't park the
                    # PE in front of fresh work); s0b of the previous pair
                    # then follows every ps2 writer.
                    while emit_wave(prev_chain):
                        pass
                if tail_stages:
                    fn = tail_stages.pop(0)
                    if fn is not None:
                        fn()

                pend.append(done)
                if len(pend) > LAG:
                    chain.avail.update(pend.pop(0))

                if s == 1:
                    # h-accumulator; allocated after s0b of the previous pair
                    # (slot 0) so the bufs=1 dependency is already emitted
                    ps2 = ps2p.tile([128, PAIR_N], f32, tag="ps2",
                                    name=f"ps2_{p}")
                    chain.ps2 = ps2
                if s >= 2:
                    emit_wave(chain)
                    if s >= 8:
                        emit_wave(chain)

            for dn in pend:
                chain.avail.update(dn)

            for st_fn in tail_stages:
                if st_fn is not None:
                    st_fn()
            tail_stages = make_tail_stages(p, ps2)
            prev_chain = chain

        while emit_wave(prev_chain):
            pass
        for st_fn in tail_stages:
            if st_fn is not None:
                st_fn()

    nc.compile()
    return nc


# ---------------------------------------------------------------- execution
_NC_CACHE = {}
LAST_RESULT = None


def _prep_inputs(inputs):
    board = np.ascontiguousarray(np.asarray(inputs["board"], np.float32))
    x = board.reshape(B_TOTAL, 64)
    dev = _fold_params(inputs)
    in_maps = []
    for c in range(N_CORES):
        xc = np.ascontiguousarray(x[c * BC:(c + 1) * BC].T)      # [64, BC]
        m = dict(dev)
        m["xx"] = np.ascontiguousarray(
            np.vstack([xc, xc]).astype(np.float16))              # [128, BC]
        in_maps.append(m)
    return in_maps


def kernel(**inputs):
    global LAST_RESULT
    from concourse.bass_utils import run_bass_kernel_spmd

    if "nc" not in _NC_CACHE:
        _NC_CACHE["nc"] = _build_nc()
    nc = _NC_CACHE["nc"]

    in_maps = _prep_inputs(inputs)
    res = run_bass_kernel_spmd(nc, in_maps, core_ids=list(range(N_CORES)))
    LAST_RESULT = res
    out = np.concatenate([r["o"].reshape(-1) for r in res.results])
    return out.reshape(B_TOTAL, 1).astype(np.float32)


# revision 34
# speedup vs baseline: 1.0073x; 1.0073x over previous
"""Trainium2 Bass kernel for nn_BlockBlastValueNet1PmultikernelFlattenned.

Strategy (v2)
-------------
The network is 8 tiny conv branches over an 8x8 board followed by small MLPs.
Every conv branch (pad const 1.0 + valid conv + bias) is an affine map of the
64 board values, so the whole net folds into:

    y  = x @ W1                          # [B, NF]  (NF = 2944, no bias in psum)
    h  = W12 x + W2s.T ev(y) + b2f       # ev = per-tile relu-ish evacuation
    g1 = Lrelu( h @ W3 + b3 )
    g2 = Lrelu( g1 @ W4 + b4 )           # augmented with a ones column
    out = g2 @ W5                        # fc3 (bias folded via augmentation)

Key structure vs v1: W2 is BLOCK-DIAGONAL (8 branches x 16 h-outputs).  The
step-2 contraction runs as 4 independent chains in the four 32-column strips
of the PE array (tile_position=(0|64, 32g)); the 4 chains execute
CONCURRENTLY, so step-2 costs ~max-chain (9 streams) instead of 23+ serial
K-tile streams.  Branch pairs are laid out in 64-row-aligned K-groups so a
K-tile shared by two groups splits at the array row-half boundary (top half
one strip, bottom half another - still concurrent).

The Lrelu between the two big matmuls is decomposed as
    Lrelu(v + c1) = 0.01 v + 0.99 ev + const
where the evacuated tensor ev is per-tile either
    r = relu(y + c1)        (Scalar/ACT engine: 1-op activation w/ bias)
    z = max(y, -c1)         (Vector/DVE engine: 1-op tensor_scalar_max)
(z = r - c1, so the difference folds into the h bias b2f with per-tile
coefficient kappa: 0.01 for r-tiles, 1.00 for z-tiles).  The 0.01*v path
collapses into a 64->128 fold matmul (W12) opening each pair's h-accumulator
(single full-width start=True clears the psum bank; the 4 strip chains then
accumulate start=False).

PSUM->SBUF evacuation is the hard wall: only DVE and ACT can read PSUM
(GPSIMD has no PSUM port on trn2, DMA none either), both at 1 elem/cycle/
partition for fp32 sources.  23 tiles x [128,1024] per pair ~= 14us across
both engines; everything else is scheduled to hide under that.

Data-parallel over 8 NeuronCores (batch 65536 -> 8192/core), feature-major
layout, samples processed in pairs of 512-chunks (psum-bank sized).  All
matmuls fp16 (full PE rate, keeps the PE activity monitor warm).
"""

import numpy as np

# ---------------------------------------------------------------- constants
SPECS = [(1, 1, 1, 0, 0), (2, 2, 6, 1, 1), (3, 3, 8, 1, 1), (4, 4, 8, 2, 2),
         (5, 5, 16, 2, 2), (8, 8, 32, 0, 0), (1, 8, 4, 0, 0), (8, 1, 4, 0, 0)]
BOARD = 8
B_TOTAL = 65536
N_CORES = 8
BC = B_TOTAL // N_CORES          # 8192 samples per core
PAIR_N = 1024                    # samples per pair-iteration (2 psum banks)
CHUNK = 512                      # matmul moving width (1 psum bank fp32)
N_PAIRS = BC // PAIR_N           # 8
LRELU_NEG = 0.01

# column-group pairing of branches (2 branches x 16 h-outputs = 32 cols each)
GROUPS = [[4, 5], [3, 6], [2, 7], [1, 0]]
_BR_N = []
for kh, kw, fs, ph, pw in SPECS:
    _BR_N.append((BOARD + 2 * ph - kh + 1) * (BOARD + 2 * pw - kw + 1) * fs)

# 64-aligned group K-offsets so straddle tiles split at the row-half boundary
G_OFF, G_END = [], []
_o = 0
for g in GROUPS:
    G_OFF.append(_o)
    _o += sum(_BR_N[b] for b in g)
    G_END.append(_o)
    _o = -(-_o // 64) * 64       # round up to 64
KT = -(-_o // 128)               # 23 K-tiles
NF = KT * 128                    # 2944

# per-group stream lists: (tile, kind) with kind in {'full','top','bot'}
G_STREAMS = []
for gi in range(4):
    a = G_OFF[gi]
    b = -(-G_END[gi] // 64) * 64
    st = []
    for t in range(a // 128, -(-b // 128)):
        lo, hi = max(a, 128 * t), min(b, 128 * t + 128)
        if hi - lo == 128:
            st.append((t, 'full'))
        elif lo == 128 * t:
            st.append((t, 'top'))
        else:
            st.append((t, 'bot'))
    G_STREAMS.append(st)
TILE_STREAMS = {}                # tile -> [(group, kind)]
for gi in range(4):
    for t, kind in G_STREAMS[gi]:
        TILE_STREAMS.setdefault(t, []).append((gi, kind))
# per-strip fold openers (the 0.01*x path): first stream of each chain,
# start=True clears exactly the partitions the chain accumulates into.
# Row-half alternates so openers of adjacent strips are fully disjoint.
FOLD_ROW = [64, 0, 64, 0]
N_STREAMS = sum(len(s) for s in G_STREAMS) + 4   # 25 + folds

# step-1 slot -> (psA tile, psB tile).  psA tiles evacuate on DVE (z-form),
# psB tiles on ACT (r-form).  Order interleaves groups so the 4 step-2
# chains get tiles round-robin.
PSA = [0, 14, 1, 15, 2, 16, 3, 17, 4, 18, 6, 7]
PSB = [9, 19, 10, 20, 11, 21, 12, 22, 13, 5, 8, None]
N_S1 = len(PSA)                  # 12
assert sorted([t for t in PSA + PSB if t is not None]) == list(range(KT))
DVE_TILES = set(PSA)
LAG = 2                          # slots between evac emission and chain use

# consume streams in tile-production order (chain accumulation commutes)
_PROD = {}
for _s in range(N_S1):
    _PROD[PSA[_s]] = _s
    if PSB[_s] is not None:
        _PROD[PSB[_s]] = _s
for _gi in range(4):
    G_STREAMS[_gi].sort(key=lambda tk: _PROD[tk[0]])
    G_STREAMS[_gi].insert(0, (None, 'fold'))   # opener; no y-tile dep


# ---------------------------------------------------------------- host fold
def _fold_params(p):
    """Fold conv branches + MLPs into the dense pipeline weights (float64)."""
    n_of = _BR_N
    W1_of, c1_of = {}, {}
    for i, (kh, kw, fs, ph, pw) in enumerate(SPECS):
        Ho = BOARD + 2 * ph - kh + 1
        Wo = BOARD + 2 * pw - kw + 1
        cw = np.asarray(p[f"b{i}_cw"], np.float64)
        cb = np.asarray(p[f"b{i}_cb"], np.float64)
        W1 = np.zeros((64, n_of[i]))
        c1 = np.zeros((n_of[i],))
        for f in range(fs):
            for oh in range(Ho):
                for ow in range(Wo):
                    oi = (f * Ho + oh) * Wo + ow
                    c1[oi] += cb[f]
                    for u in range(kh):
                        for v in range(kw):
                            r, c = oh + u - ph, ow + v - pw
                            w = cw[f, 0, u, v]
                            if 0 <= r < 8 and 0 <= c < 8:
                                W1[r * 8 + c, oi] += w
                            else:
                                c1[oi] += w        # pad value is 1.0
        W1_of[i] = W1
        c1_of[i] = c1

    # K-layout: 64-aligned groups; h block order = group order
    K_start = {}
    for gi, g in enumerate(GROUPS):
        off = G_OFF[gi]
        for b in g:
            K_start[b] = off
            off += n_of[b]
    border = [b for g in GROUPS for b in g]
    hpos = {b: j * 16 for j, b in enumerate(border)}

    W1p = np.zeros((64, NF))
    c1p = np.zeros((NF,))
    W2p = np.zeros((NF, 128))
    b2p = np.zeros((128,))
    for b in range(8):
        s, n, hp = K_start[b], n_of[b], hpos[b]
        W1p[:, s:s + n] = W1_of[b]
        c1p[s:s + n] = c1_of[b]
        W2p[s:s + n, hp:hp + 16] = np.asarray(p[f"b{b}_w1"], np.float64).T
        b2p[hp:hp + 16] = np.asarray(p[f"b{b}_b1"], np.float64)

    Wb = np.zeros((128, 64))
    bb = np.zeros((64,))
    for b in range(8):
        hp = hpos[b]
        Wb[hp:hp + 16, 8 * b:8 * b + 8] = np.asarray(p[f"b{b}_w2"], np.float64).T
        bb[8 * b:8 * b + 8] = np.asarray(p[f"b{b}_b2"], np.float64)
    fc_w1 = np.asarray(p["fc_w1"], np.float64)
    fc_b1 = np.asarray(p["fc_b1"], np.float64)
    W3 = Wb @ fc_w1.T
    b3 = bb @ fc_w1.T + fc_b1
    fc_w2 = np.asarray(p["fc_w2"], np.float64)
    fc_b2 = np.asarray(p["fc_b2"], np.float64)
    fc_w3 = np.asarray(p["fc_w3"], np.float64)
    fc_b3 = np.asarray(p["fc_b3"], np.float64)
    W4 = np.zeros((64, 17)); W4[:, :16] = fc_w2.T
    b4 = np.zeros((17,)); b4[:16] = fc_b2; b4[16] = 1.0
    W5 = np.zeros((17, 1)); W5[:16, 0] = fc_w3[0]; W5[16, 0] = fc_b3[0]

    # relu decomposition folds: Lrelu(y+c1) = 0.01(y+c1) + 0.99 relu(y+c1)
    # DVE tiles evacuate z = max(y,-c1) = relu(y+c1) - c1  -> kappa = 1.00
    # ACT tiles evacuate r = relu(y+c1)                    -> kappa = 0.01
    W2s = (1.0 - LRELU_NEG) * W2p
    W12 = LRELU_NEG * (W1p @ W2p)
    b2f = b2p.copy()
    for t in range(KT):
        kap = 1.0 if t in DVE_TILES else LRELU_NEG
        b2f += kap * (c1p[128 * t:128 * (t + 1)] @ W2p[128 * t:128 * (t + 1)])

    f32 = np.float32
    f16 = np.float16
    dev = {}
    # step-1 weights: slot s holds M-tiles PSA[s] (rows 0:64) | PSB[s] (64:128)
    w1 = np.zeros((128, N_S1, 128), f16)
    for s in range(N_S1):
        w1[0:64, s, :] = W1p[:, 128 * PSA[s]:128 * (PSA[s] + 1)]
        if PSB[s] is not None:
            w1[64:128, s, :] = W1p[:, 128 * PSB[s]:128 * (PSB[s] + 1)]
    dev["w1"] = w1
    c1t = np.zeros((128, KT), f32)      # ACT bias (+c1)
    nc1t = np.zeros((128, KT), f32)     # DVE max operand (-c1)
    for t in range(KT):
        c1t[:, t] = c1p[128 * t:128 * (t + 1)]
        nc1t[:, t] = -c1p[128 * t:128 * (t + 1)]
    dev["c1t"] = c1t
    dev["nc1t"] = nc1t
    w2 = np.zeros((128, KT, 128), f16)
    for t in range(KT):
        w2[:, t, :] = W2s[128 * t:128 * (t + 1), :]
    dev["w2"] = w2
    dev["w12"] = np.vstack([W12, W12]).astype(f16)  # both row-halves
    dev["b2f"] = b2f.reshape(128, 1).astype(f32)
    dev["w3"] = W3.astype(f16)
    dev["b3"] = b3.reshape(64, 1).astype(f32)
    dev["w4"] = W4.astype(f16)
    dev["b4"] = b4.reshape(17, 1).astype(f32)
    dev["w5"] = W5.astype(f16)
    return dev


# ---------------------------------------------------------------- device IR
def _build_nc(n_pairs=N_PAIRS):
    import concourse.mybir as mybir
    import concourse.tile as tile
    from concourse import bacc
    from contextlib import ExitStack

    dt = mybir.dt
    AF = mybir.ActivationFunctionType
    f32 = dt.float32
    f16 = dt.float16
    bc = n_pairs * PAIR_N

    nc = bacc.Bacc("TRN2", target_bir_lowering=False, debug=False,
                   num_devices=N_CORES)

    xx_d = nc.dram_tensor("xx", [128, bc], f16, kind="ExternalInput")
    w1_d = nc.dram_tensor("w1", [128, N_S1, 128], f16, kind="ExternalInput")
    c1t_d = nc.dram_tensor("c1t", [128, KT], f32, kind="ExternalInput")
    nc1t_d = nc.dram_tensor("nc1t", [128, KT], f32, kind="ExternalInput")
    w2_d = nc.dram_tensor("w2", [128, KT, 128], f16, kind="ExternalInput")
    w12_d = nc.dram_tensor("w12", [128, 128], f16, kind="ExternalInput")
    b2f_d = nc.dram_tensor("b2f", [128, 1], f32, kind="ExternalInput")
    w3_d = nc.dram_tensor("w3", [128, 64], f16, kind="ExternalInput")
    b3_d = nc.dram_tensor("b3", [64, 1], f32, kind="ExternalInput")
    w4_d = nc.dram_tensor("w4", [64, 17], f16, kind="ExternalInput")
    b4_d = nc.dram_tensor("b4", [17, 1], f32, kind="ExternalInput")
    w5_d = nc.dram_tensor("w5", [17, 1], f16, kind="ExternalInput")
    o_d = nc.dram_tensor("o", [1, bc], f32, kind="ExternalOutput")

    with tile.TileContext(nc) as tc, ExitStack() as ctx:
        wpool = ctx.enter_context(tc.tile_pool(name="wpool", bufs=1))
        xpool = ctx.enter_context(tc.tile_pool(name="xpool", bufs=3))
        ypool = ctx.enter_context(tc.tile_pool(name="ypool", bufs=KT + 3))
        spool = ctx.enter_context(tc.tile_pool(name="spool", bufs=2))
        ps1p = ctx.enter_context(tc.tile_pool(name="ps1p", bufs=3, space="PSUM"))
        ps2p = ctx.enter_context(tc.tile_pool(name="ps2p", bufs=1, space="PSUM"))

        # pair-0 input first so compute can start while the rest streams in
        xx_first = xpool.tile([128, PAIR_N], f16, tag="xx", name="xx_first")
        nc.sync.dma_start(xx_first[:], xx_d[:, 0:PAIR_N])
        w1_t = wpool.tile([128, N_S1, 128], f16)
        nc.gpsimd.dma_start(w1_t[:], w1_d[:])
        c1t_t = wpool.tile([128, KT], f32)
        nc.gpsimd.dma_start(c1t_t[:], c1t_d[:])
        nc1t_t = wpool.tile([128, KT], f32)
        nc.gpsimd.dma_start(nc1t_t[:], nc1t_d[:])
        w2_t = wpool.tile([128, KT, 128], f16)
        nc.gpsimd.dma_start(w2_t[:], w2_d[:])
        w12_t = wpool.tile([128, 128], f16)
        nc.gpsimd.dma_start(w12_t[:], w12_d[:])
        b2f_t = wpool.tile([128, 1], f32)
        nc.gpsimd.dma_start(b2f_t[:], b2f_d[:])
        w3_t = wpool.tile([128, 64], f16)
        nc.gpsimd.dma_start(w3_t[:], w3_d[:])
        b3_t = wpool.tile([64, 1], f32)
        nc.gpsimd.dma_start(b3_t[:], b3_d[:])
        w4_t = wpool.tile([64, 17], f16)
        nc.gpsimd.dma_start(w4_t[:], w4_d[:])
        b4_t = wpool.tile([17, 1], f32)
        nc.gpsimd.dma_start(b4_t[:], b4_d[:])
        w5_t = wpool.tile([17, 1], f16)
        nc.gpsimd.dma_start(w5_t[:], w5_d[:])

        # PE warm-up: dependency-free dummy matmuls run while the input
        # DMAs land, so the HAM un-throttles (K=8/8 @ 2.4GHz) before the
        # first real slot instead of ~4us into it.  Results are discarded
        # (the scratch psum is never read; real slots start=True overwrite).
        scratch = wpool.tile([128, CHUNK], f16, name="warmup_src")
        nc.vector.memset(scratch[:], 0)
        wu_ps = ps1p.tile([128, PAIR_N], f32, tag="ps1", name="warmup_ps")
        for _wu in range(32):
            nc.tensor.matmul(wu_ps[:, 0:128], scratch[0:64, 0:128],
                             scratch[0:64, 0:128], start=True, stop=True,
                             tile_position=(0, 0))

        class ChainState:
            """Per-pair step-2 chain bookkeeping (survives into next pair)."""
            def __init__(self, ps2, ytiles, xx):
                self.ps2 = ps2
                self.ytiles = ytiles
                self.xx = xx
                self.q = [list(s) for s in G_STREAMS]  # pending per group
                self.avail = set()                     # tiles evac'd (lagged)
                self.left = N_STREAMS

            def ready(self, gi):
                if not self.q[gi]:
                    return False
                t = self.q[gi][0][0]
                return t is None or t in self.avail

            def empty(self):
                return not any(self.q)

        def emit_wave(st):
            """Emit one concurrent wave: <=1 stream per group, interleaved
            chunks so the 4 col-strips run concurrently on the PE."""
            popped = []
            for gi in range(4):
                if st.ready(gi):
                    popped.append((gi,) + st.q[gi].pop(0))
            if not popped:
                return False
            st.left -= len(popped)
            last = st.left == 0
            for h in range(2):
                sl = slice(h * CHUNK, (h + 1) * CHUNK)
                for i, (gi, t, kind) in enumerate(popped):
                    cs = slice(32 * gi, 32 * gi + 32)
                    stop = last and i == len(popped) - 1
                    if kind == 'fold':
                        r0 = FOLD_ROW[gi]
                        nc.tensor.matmul(
                            st.ps2[cs, sl], w12_t[r0:r0 + 64, cs],
                            st.xx[r0:r0 + 64, sl], start=True, stop=stop,
                            tile_position=(r0, 32 * gi),
                            skip_group_check=True)
                    elif kind == 'full':
                        nc.tensor.matmul(
                            st.ps2[cs, sl], w2_t[:, t, cs],
                            st.ytiles[t][:, sl], start=False, stop=stop,
                            tile_position=(0, 32 * gi), skip_group_check=True)
                    elif kind == 'top':
                        nc.tensor.matmul(
                            st.ps2[cs, sl], w2_t[0:64, t, cs],
                            st.ytiles[t][0:64, sl], start=False, stop=stop,
                            tile_position=(0, 32 * gi), skip_group_check=True)
                    else:
                        nc.tensor.matmul(
                            st.ps2[cs, sl], w2_t[64:128, t, cs],
                            st.ytiles[t][64:128, sl], start=False, stop=stop,
                            tile_position=(64, 32 * gi), skip_group_check=True)
            return True

        def make_tail_stages(p, ps2):
            """Per-pair serial tail (h -> g1 -> g2 -> out), interleaved into
            the NEXT pair's slot stream."""
            st = {}

            def s0b():
                st["h"] = spool.tile([128, PAIR_N], f16, tag="h", name=f"h_{p}")
                nc.scalar.activation(st["h"][:], ps2[:], AF.Lrelu,
                                     bias=b2f_t[:, 0:1], alpha=LRELU_NEG)

            def s1():
                st["g1ps"] = ps1p.tile([64, PAIR_N], f32, tag="ps1",
                                       name=f"g1ps_{p}")
                for h in range(2):
                    sl = slice(h * CHUNK, (h + 1) * CHUNK)
                    nc.tensor.matmul(st["g1ps"][0:32, sl], w3_t[:, 0:32],
                                     st["h"][:, sl], start=True, stop=True,
                                     tile_position=(0, 0),
                                     skip_group_check=True)
                    nc.tensor.matmul(st["g1ps"][32:64, sl], w3_t[:, 32:64],
                                     st["h"][:, sl], start=True, stop=True,
                                     tile_position=(0, 32),
                                     skip_group_check=True)

            def s2():
                st["g1"] = spool.tile([64, PAIR_N], f16, tag="g1",
                                      name=f"g1_{p}")
                nc.scalar.activation(st["g1"][:], st["g1ps"][:], AF.Lrelu,
                                     bias=b3_t[:, 0:1], alpha=LRELU_NEG)

            def s3():
                # g2ps and the final output row share one psum tile
                st["tps"] = ps1p.tile([33, PAIR_N], f32, tag="ps1",
                                      name=f"tps_{p}")
                for h in range(2):
                    sl = slice(h * CHUNK, (h + 1) * CHUNK)
                    nc.tensor.matmul(st["tps"][0:17, sl], w4_t[:],
                                     st["g1"][:, sl], start=True, stop=True)

            def s4():
                st["g2"] = spool.tile([17, PAIR_N], f16, tag="g2",
                                      name=f"g2_{p}")
                nc.scalar.activation(st["g2"][:], st["tps"][0:17, :], AF.Lrelu,
                                     bias=b4_t[:, 0:1], alpha=LRELU_NEG)

            def s5():
                for h in range(2):
                    sl = slice(h * CHUNK, (h + 1) * CHUNK)
                    nc.tensor.matmul(st["tps"][32:33, sl], w5_t[:],
                                     st["g2"][:, sl], start=True, stop=True,
                                     tile_position=(0, 32),
                                     skip_group_check=True)

            def s6():
                o_t = spool.tile([1, PAIR_N], f32, tag="o", name=f"o_{p}")
                nc.vector.tensor_copy(o_t[:, 0:CHUNK],
                                      st["tps"][32:33, 0:CHUNK])
                nc.scalar.activation(o_t[:, CHUNK:PAIR_N],
                                     st["tps"][32:33, CHUNK:PAIR_N], AF.Copy)
                nc.sync.dma_start(o_d[:, p * PAIR_N:(p + 1) * PAIR_N], o_t[:])

            return [s0b, None, s1, s2, None, s3, s4, None, s5, s6]

        tail_stages = []
        prev_chain = None

        for p in range(n_pairs):
            if p == 0:
                xx_t = xx_first
            else:
                xx_t = xpool.tile([128, PAIR_N], f16, tag="xx", name=f"xx_{p}")
                nc.sync.dma_start(xx_t[:],
                                  xx_d[:, p * PAIR_N:(p + 1) * PAIR_N])

            ytiles = [None] * KT
            chain = ChainState(None, ytiles, xx_t)

            pend = []
            for s in range(N_S1):
                tA, tB = PSA[s], PSB[s]
                psA = ps1p.tile([128, PAIR_N], f32, tag="ps1",
                                name=f"psA_{p}_{s}")
                if tB is not None:
                    psB = ps1p.tile([128, PAIR_N], f32, tag="ps1",
                                    name=f"psB_{p}_{s}")
                for h in range(2):
                    sl = slice(h * CHUNK, (h + 1) * CHUNK)
                    nc.tensor.matmul(psA[:, sl], w1_t[0:64, s, :],
                                     xx_t[0:64, sl], start=True, stop=True,
                                     tile_position=(0, 0))
                    if tB is not None:
                        nc.tensor.matmul(psB[:, sl], w1_t[64:128, s, :],
                                         xx_t[64:128, sl], start=True,
                                         stop=True, tile_position=(64, 0))

                # evacuations first so they are never queued behind a
                # ~1.1us tail-stage op on the same engine
                yA = ypool.tile([128, PAIR_N], f16, tag="y", name=f"y_{p}_{tA}")
                nc.vector.tensor_scalar_max(yA[:], psA[:],
                                            nc1t_t[:, tA:tA + 1])
                ytiles[tA] = yA
                done = [tA]
                if tB is not None:
                    yB = ypool.tile([128, PAIR_N], f16, tag="y",
                                    name=f"y_{p}_{tB}")
                    nc.scalar.activation(yB[:], psB[:], AF.Relu,
                                         bias=c1t_t[:, tB:tB + 1])
                    ytiles[tB] = yB
                    done.append(tB)

                if s == 0 and prev_chain is not None:
                    # finish previous pair's chains (queued behind this
                    # slot's step-1 MMs, so their evac waits don't park the
                    # PE in front of fresh work); s0b of the previous pair
                    # then follows every ps2 writer.
                    while emit_wave(prev_chain):
                        pass
                if tail_stages:
                    fn = tail_stages.pop(0)
                    if fn is not None:
                        fn()

                pend.append(done)
                if len(pend) > LAG:
                    chain.avail.update(pend.pop(0))

                if s == 1:
                    # h-accumulator; allocated after s0b of the previous pair
                    # (slot 0) so the bufs=1 dependency is already emitted
                    ps2 = ps2p.tile([128, PAIR_N], f32, tag="ps2",
                                    name=f"ps2_{p}")
                    chain.ps2 = ps2
                if s >= 2:
                    emit_wave(chain)
                    if s >= 8:
                        emit_wave(chain)

            for dn in pend:
                chain.avail.update(dn)

            for st_fn in tail_stages:
                if st_fn is not None:
                    st_fn()
            tail_stages = make_tail_stages(p, ps2)
            prev_chain = chain

        while emit_wave(prev_chain):
            pass
        for st_fn in tail_stages:
            if st_fn is not None:
                st_fn()

    nc.compile()
    return nc


# ---------------------------------------------------------------- execution
_NC_CACHE = {}
LAST_RESULT = None


def _prep_inputs(inputs):
    board = np.ascontiguousarray(np.asarray(inputs["board"], np.float32))
    x = board.reshape(B_TOTAL, 64)
    dev = _fold_params(inputs)
    in_maps = []
    for c in range(N_CORES):
        xc = np.ascontiguousarray(x[c * BC:(c + 1) * BC].T)      # [64, BC]
        m = dict(dev)
        m["xx"] = np.ascontiguousarray(
            np.vstack([xc, xc]).astype(np.float16))              # [128, BC]
        in_maps.append(m)
    return in_maps


def kernel(**inputs):
    global LAST_RESULT
    from concourse.bass_utils import run_bass_kernel_spmd

    if "nc" not in _NC_CACHE:
        _NC_CACHE["nc"] = _build_nc()
    nc = _NC_CACHE["nc"]

    in_maps = _prep_inputs(inputs)
    res = run_bass_kernel_spmd(nc, in_maps, core_ids=list(range(N_CORES)))
    LAST_RESULT = res
    out = np.concatenate([r["o"].reshape(-1) for r in res.results])
    return out.reshape(B_TOTAL, 1).astype(np.float32)
